# revision 1
# baseline (speedup 1.0000x reference)
"""Trainium2 Bass kernel for nn_ConstLoss_22746146800082 (factorized).

loss = mean_{i != j} (Cq[i,j] - Ck[i,j])^2 with Cx the pairwise cosine
matrix of feat_x (N=4096, D=1024).  The eps terms in the reference cancel,
so Cx is the cosine matrix of the raw rows, and the diagonal of Cq - Ck is
~0, so the mask reduces to a constant denominator.

Factorization: ||Cq - Ck||_F^2 = ||Aqq||^2 + ||Akk||^2 - 2 ||Aqk||^2 with
feature-space Grams Aqq = Q^T Q, Akk = K^T K, Aqk = K^T Q of the
row-normalized features (1024x1024 each) - 2.7x fewer MACs than forming
the 4096x4096 similarity matrices.  Row normalization folds into the
stationary operand only (Aqq = (rq^2 . Q)^T Q etc.), so the streamed
moving operand stays raw bf16.

Sharding: output features are sharded across 8 cores (128 each); every
core streams all N samples (natural layout, bf16) and contracts them into
its [128, 1024] slice of all three Grams, held in 6 PSUM banks across the
whole contraction.  Row norms: each core computes fp32 norms of its own
512 rows from the same bf16 data (this cancels the radial part of the
bf16 input rounding; measured 2e-8 end-to-end) and AllGathers the 4KB of
inverse norms.  Each core reduces its Gram slices to one scalar; the host
sums the 8 partials.
"""

import numpy as np

import concourse.bass as bass
import concourse.mybir as mybir
import concourse.tile as tile
from concourse.vector_clock import ScopedClock
from concourse.bass_utils import run_bass_kernel_spmd

N_CORES = 8
N = 4096
D = 1024
P = 128

B = N // N_CORES          # own rows per core (512)
NC = N // P               # sample chunks (32)
MG = 4                    # chunks merged per DMA
NM = NC // MG             # merged groups (8)

F32 = mybir.dt.float32
BF16 = mybir.dt.bfloat16
ACTF = mybir.ActivationFunctionType


class _TC(tile.TileContext):
    """TileContext whose kernel-tail drain splits its semaphore waits across
    preceding sync-engine NOPs: this container's walrus build rejects a Drain
    carrying more than one sync wait ("Too many sync wait commands")."""

    def _drain_and_barrier(self, tick_clock, wait_clock):
        nc = self.nc
        probe = nc.sync.nop(nofuse=True)
        wait_clock.add_sem_waits(
            probe.ins, ScopedClock({None: tick_clock.global_clock})
        )
        waits = list(probe.ins.sync_info.on_wait or []) if probe.ins.sync_info else []
        if probe.ins.sync_info is not None:
            probe.ins.sync_info.on_wait = waits[:1]
        for w in waits[1:]:
            n2 = nc.sync.nop(nofuse=True)
            n2.ins.sync_info = mybir.SyncInfo(on_wait=[w], on_update=[])
        nc.sync.drain()
        nc.all_engine_barrier()
        popped = nc._tile_sem_poison_stack.pop()
        assert popped is self._sem_poison
        nc.clear_and_free_semaphores(list(self.sems.allocated().values()))
        nc.all_engine_barrier()


MAX_WAITS_PER_INST = 1


def split_excess_waits(nc):
    """walrus (this build) rejects instructions carrying more than a couple
    of semaphore waits.  Hoist excess waits onto injected same-engine NOPs
    placed immediately before the offending instruction."""
    n = 0
    for f in nc.m.functions:
        for bb in f.blocks:
            insts = bb.instructions
            out = []
            changed = False
            for ins in insts:
                si = ins.sync_info
                waits = list(si.on_wait or []) if si is not None else []
                while len(waits) > MAX_WAITS_PER_INST:
                    take = waits[:MAX_WAITS_PER_INST]
                    waits = waits[MAX_WAITS_PER_INST:]
                    nop = mybir.InstNoOp(name=f"I-waitsplit-{n}", ins=[], outs=[])
                    n += 1
                    nop.engine = ins.engine
                    nop.sync_info = mybir.SyncInfo(on_wait=take, on_update=[])
                    out.append(nop)
                    changed = True
                if changed and si is not None:
                    si.on_wait = waits
                out.append(ins)
            if changed:
                bb.instructions = out
    return n


def build_program(sim_mode: bool = False):
    nc = bass.Bass(
        "TRN2", target_bir_lowering=False, debug=False, num_devices=N_CORES
    )
    fq16 = nc.dram_tensor("fq16", [N, D], BF16, kind="ExternalInput").ap()
    fk16 = nc.dram_tensor("fk16", [N, D], BF16, kind="ExternalInput").ap()
    # per-core feature-block column slices (raw bf16)
    fqa = nc.dram_tensor("fqa", [N, P], BF16, kind="ExternalInput").ap()
    fka = nc.dram_tensor("fka", [N, P], BF16, kind="ExternalInput").ap()
    # per-core own 512 rows (same bf16 values as the stream)
    fqn = nc.dram_tensor("fqn", [B, D], BF16, kind="ExternalInput").ap()
    fkn = nc.dram_tensor("fkn", [B, D], BF16, kind="ExternalInput").ap()
    out = nc.dram_tensor("out", [1, 1], F32, kind="ExternalOutput").ap()

    with _TC(nc) as tc:
        with (
            tc.tile_pool(name="consts", bufs=1) as consts,
            tc.tile_pool(name="norms", bufs=1) as norms,
            tc.tile_pool(name="ntmp", bufs=2) as ntmp,
            tc.tile_pool(name="stream", bufs=5) as stream,
            tc.tile_pool(name="ablk", bufs=5) as ablk,
            tc.tile_pool(name="ltile", bufs=6) as ltile,
            tc.tile_pool(name="rows", bufs=1) as rows,
            tc.tile_pool(name="psum", bufs=1, space="PSUM") as psum,
            tc.tile_pool(name="psum_f", bufs=1, space="PSUM") as psum_f,
            tc.tile_pool(name="dram", bufs=1, space="DRAM") as dram,
        ):
            ones = consts.tile([P, 1], F32)
            nc.vector.memset(ones, 1.0)

            # ---- own-row norms -> AllGather inverse norms -----------------
            cc_in = dram.tile([2, B], F32)
            for mi, fn_ in enumerate((fqn, fkn)):
                n4 = norms.tile([P, MG, D], BF16, name="n4", tag="n4", bufs=2)
                nc.sync.dma_start(
                    out=n4, in_=fn_.rearrange("(s p) d -> p s d", p=P)
                )
                n2c = ntmp.tile([P, MG], F32, name="n2c", tag="n2c")
                for s in range(MG):
                    tr = ntmp.tile([P, D], F32, name="tr", tag="tr")
                    nc.vector.tensor_mul(tr, n4[:, s, :], n4[:, s, :])
                    nc.vector.tensor_reduce(
                        n2c[:, s : s + 1], tr,
                        axis=mybir.AxisListType.X, op=mybir.AluOpType.add,
                    )
                dst = bass.AP(
                    cc_in.tensor, cc_in.offset + mi * B, [[1, P], [P, MG]]
                )
                nc.gpsimd.dma_start(out=dst, in_=n2c)

            cc_out = dram.tile(
                [2 * N_CORES, B], F32,
                addr_space="Local" if sim_mode else "Shared",
            )
            if sim_mode:
                for c in range(N_CORES):
                    nc.gpsimd.dma_start(
                        out=cc_out[2 * c : 2 * c + 2, :], in_=cc_in
                    )
            else:
                nc.gpsimd.collective_compute(
                    "AllGather",
                    mybir.AluOpType.bypass,
                    replica_groups=[list(range(N_CORES))],
                    ins=[cc_in.opt()],
                    outs=[cc_out.opt()],
                )

            # all-rows inverse norms as [P, NM, MG]: element (p, g, s) =
            # rinv[global row (g*MG + s)*128 + p]
            # cc_out q rows at element offset 1024*c + (gl%4)*128 + p, where
            # global chunk gl = g*MG+s maps to core c = gl//4, slot gl%4.
            rivq = norms.tile([P, NM, MG], F32, name="rivq")
            rivk = norms.tile([P, NM, MG], F32, name="rivk")
            for t, base in ((rivq, 0), (rivk, B)):
                for g in range(NM):
                    nc.gpsimd.dma_start(
                        out=t[:, g, :],
                        in_=bass.AP(
                            cc_out.tensor,
                            cc_out.offset + base + g * 2 * B,
                            [[1, P], [P, MG]],
                        ),
                    )
            # rivq/rivk hold the gathered n^2 values; self-weights are the
            # exact DVE reciprocals, the cross-weight is sqrt(wqq*wkk).
            wqq3 = norms.tile([P, NM, MG], F32, name="wqq3")
            wkk3 = norms.tile([P, NM, MG], F32, name="wkk3")
            wqk3 = norms.tile([P, NM, MG], F32, name="wqk3")
            t3 = norms.tile([P, NM, MG], F32, name="t3")
            nc.vector.reciprocal(wqq3, rivq)
            nc.vector.reciprocal(wkk3, rivk)
            nc.vector.tensor_mul(t3, wqq3, wkk3)
            nc.scalar.sqrt(wqk3, t3)

            # ---- contraction: 6 PSUM banks across all 32 chunks -----------
            ps = {}
            for g_ in ("qq", "kk", "qk"):
                for h in range(2):
                    ps[(g_, h)] = psum.tile(
                        [P, 512], F32, name=f"ps_{g_}{h}", tag=f"ps_{g_}{h}"
                    )

            for g in range(NM):
                sq4 = stream.tile([P, MG, D], BF16, name="sq4", tag="sq4")
                sk4 = stream.tile([P, MG, D], BF16, name="sk4", tag="sk4")
                nc.sync.dma_start(
                    out=sq4,
                    in_=fq16.rearrange("(g s p) d -> g p s d", s=MG, p=P)[g],
                )
                nc.sync.dma_start(
                    out=sk4,
                    in_=fk16.rearrange("(g s p) d -> g p s d", s=MG, p=P)[g],
                )
                aq4 = ablk.tile([P, MG, P], BF16, name="aq4", tag="aq4")
                ak4 = ablk.tile([P, MG, P], BF16, name="ak4", tag="ak4")
                nc.sync.dma_start(
                    out=aq4,
                    in_=fqa.rearrange("(g s p) a -> g p s a", s=MG, p=P)[g],
                )
                nc.sync.dma_start(
                    out=ak4,
                    in_=fka.rearrange("(g s p) a -> g p s a", s=MG, p=P)[g],
                )
                for s in range(MG):
                    ci = g * MG + s
                    lqq = ltile.tile([P, P], BF16, name="lqq", tag="lqq")
                    lkk = ltile.tile([P, P], BF16, name="lkk", tag="lkk")
                    lqk = ltile.tile([P, P], BF16, name="lqk", tag="lqk")
                    nc.vector.tensor_scalar_mul(
                        lqq, aq4[:, s, :], wqq3[:, g, s : s + 1]
                    )
                    nc.vector.tensor_scalar_mul(
                        lkk, ak4[:, s, :], wkk3[:, g, s : s + 1]
                    )
                    nc.vector.tensor_scalar_mul(
                        lqk, ak4[:, s, :], wqk3[:, g, s : s + 1]
                    )
                    st = dict(start=(ci == 0), stop=(ci == NC - 1))
                    for h in range(2):
                        hs = slice(h * 512, (h + 1) * 512)
                        nc.tensor.matmul(
                            ps[("qq", h)], lhsT=lqq, rhs=sq4[:, s, hs], **st
                        )
                        nc.tensor.matmul(
                            ps[("kk", h)], lhsT=lkk, rhs=sk4[:, s, hs], **st
                        )
                        nc.tensor.matmul(
                            ps[("qk", h)], lhsT=lqk, rhs=sq4[:, s, hs], **st
                        )

            # ---- finish: S = sum(Aqq^2) + sum(Akk^2) - 2 sum(Aqk^2) -------
            accw = consts.tile([P, 6], F32)
            for idx, key in enumerate(ps):
                cp = rows.tile([P, 512], F32, name=f"cp{idx}", tag="cp", bufs=2)
                nc.vector.tensor_copy(cp, ps[key])
                sqv = rows.tile([P, 512], F32, name=f"sqv{idx}", tag="sqv", bufs=2)
                nc.vector.tensor_mul(sqv, cp, cp)
                nc.vector.tensor_reduce(
                    accw[:, idx : idx + 1], sqv,
                    axis=mybir.AxisListType.X, op=mybir.AluOpType.add,
                )
            # red = (qq0+qq1+kk0+kk1) - 2*(qk0+qk1); ps dict order is
            # qq0,qq1,kk0,kk1,qk0,qk1
            r1 = rows.tile([P, 1], F32, name="r1")
            r2 = rows.tile([P, 1], F32, name="r2")
            nc.vector.tensor_reduce(
                r1, accw[:, 0:4], axis=mybir.AxisListType.X, op=mybir.AluOpType.add
            )
            nc.vector.tensor_reduce(
                r2, accw[:, 4:6], axis=mybir.AxisListType.X, op=mybir.AluOpType.add
            )
            red = rows.tile([P, 1], F32, name="red")
            nc.vector.tensor_scalar_mul(red, r2, -2.0)
            nc.vector.tensor_add(red, red, r1)
            pf = psum_f.tile([1, 1], F32, name="pf", tag="pf")
            nc.tensor.matmul(pf, lhsT=ones, rhs=red, start=True, stop=True)
            s_ = rows.tile([1, 1], F32, name="s_")
            nc.vector.tensor_copy(s_, pf)
            nc.sync.dma_start(out=out, in_=s_)

    split_excess_waits(nc)
    return nc


_CACHE = {}


def kernel(feat_q: np.ndarray, feat_k: np.ndarray) -> np.ndarray:
    import ml_dtypes

    fq = np.ascontiguousarray(np.asarray(feat_q, dtype=np.float32))
    fk = np.ascontiguousarray(np.asarray(feat_k, dtype=np.float32))
    assert fq.shape == (N, D) and fk.shape == (N, D)

    if "nc" not in _CACHE:
        _CACHE["nc"] = build_program()
    nc = _CACHE["nc"]

    fq16 = fq.astype(ml_dtypes.bfloat16)
    fk16 = fk.astype(ml_dtypes.bfloat16)
    in_maps = []
    for c in range(N_CORES):
        cs = slice(c * P, (c + 1) * P)
        rs = slice(c * B, (c + 1) * B)
        in_maps.append(
            {
                "fq16": fq16,
                "fk16": fk16,
                "fqa": np.ascontiguousarray(fq16[:, cs]),
                "fka": np.ascontiguousarray(fk16[:, cs]),
                "fqn": np.ascontiguousarray(fq16[rs, :]),
                "fkn": np.ascontiguousarray(fk16[rs, :]),
            }
        )
    res = run_bass_kernel_spmd(nc, in_maps, list(range(N_CORES)))
    total = np.float32(0.0)
    for c in range(N_CORES):
        total += res.results[c]["out"][0, 0]
    loss = np.float32(total / np.float32(N * (N - 1)))
    return np.asarray(loss, dtype=np.float32)


if __name__ == "__main__":
    rng = np.random.default_rng(0)
    q = rng.standard_normal((N, D)).astype(np.float32)
    k = rng.standard_normal((N, D)).astype(np.float32)
    print("loss:", kernel(q, k))



# revision 5
# speedup vs baseline: 4.2287x; 4.2287x over previous
"""Trainium2 Bass kernel for nn_ConstLoss_22746146800082 (fp8 Gram factorization).

loss = mean_{i != j} (Cq[i,j] - Ck[i,j])^2 with Cx the NxN pairwise cosine
matrix of feat_x (N=4096, D=1024).  With unit rows (eps terms cancel, diag
cancels exactly):

    loss*N*(N-1) = ||Gq||_F^2 + ||Gk||_F^2 - 2 ||Gx||_F^2

with the DxD feature Grams Gq = Qn^T Qn, Gk = Kn^T Kn, Gx = Qn^T Kn - 2.7x
fewer MACs than the NxN route.  Host combines the three norms.

Sharding: each 1024x1024 Gram is partitioned into a 2x4 grid of [512 x 256]
patches, one patch triple (Gq/Gk/Gx) per core.  Each core streams only its
column footprint (R 512 cols + C 256 cols, fp8, rows = all 4096 samples) of
Q and K; the matmul stationary operands are SBUF slices of the same stream
(no separate stationary DMA).  fp8e4 DoubleRow matmuls (K=256/pass)
accumulate each patch over 16 sample-chunks.  PSUM has 8 banks but there
are 12 accumulation groups, so banks ping-pong: Gq uses banks 0-3 (Q stream
arrives first), is squared out on ACT, then Gx reuses those banks while Gk
runs in banks 4-7.  Tail squares use fused square+row-sum on ACT (Gq, Gk)
and DVE scalar_tensor_tensor (Gx); the kernel DMAs out [128, 12]
per-partition partials and the host does the final weighted sum - no
on-device collectives.

Host prep: rows are L2-normalized in fp32, scaled by 16, quantized to
e4m3 (measured end-to-end rel err ~1.6e-3 vs fp64, gate is 2e-2).
"""

import numpy as np

import concourse.bass as bass
import concourse.mybir as mybir
import concourse.tile as tile
from concourse.vector_clock import ScopedClock
from concourse.bass_utils import run_bass_kernel_spmd

N_CORES = 8
N = 4096
D = 1024
P = 128

NCH = 16          # contraction chunks of 256 samples (2 DoubleRow k-tiles)
FP = 768          # per-core column footprint: R (512) | C (256)
RW = 512          # patch rows (stationary cols)
CW = 256          # patch cols (moving cols)
NDMA = 4          # stream DMA instructions per matrix (4 chunks each)
SCALE = 16.0

F32 = mybir.dt.float32
FP8 = mybir.dt.float8e4
DR = mybir.MatmulPerfMode.DoubleRow
ACTF = mybir.ActivationFunctionType
ALU = mybir.AluOpType


class _TC(tile.TileContext):
    """TileContext whose kernel-tail drain splits its semaphore waits across
    preceding sync-engine NOPs: this container's walrus build rejects a Drain
    carrying more than one sync wait ("Too many sync wait commands")."""

    def _drain_and_barrier(self, tick_clock, wait_clock):
        nc = self.nc
        probe = nc.sync.nop(nofuse=True)
        wait_clock.add_sem_waits(
            probe.ins, ScopedClock({None: tick_clock.global_clock})
        )
        waits = list(probe.ins.sync_info.on_wait or []) if probe.ins.sync_info else []
        if probe.ins.sync_info is not None:
            probe.ins.sync_info.on_wait = waits[:1]
        for w in waits[1:]:
            n2 = nc.sync.nop(nofuse=True)
            n2.ins.sync_info = mybir.SyncInfo(on_wait=[w], on_update=[])
        nc.sync.drain()
        nc.all_engine_barrier()
        popped = nc._tile_sem_poison_stack.pop()
        assert popped is self._sem_poison
        nc.clear_and_free_semaphores(list(self.sems.allocated().values()))
        nc.all_engine_barrier()


MAX_WAITS_PER_INST = 1


def split_excess_waits(nc):
    """walrus (this build) rejects instructions carrying more than a couple
    of semaphore waits.  Hoist excess waits onto injected same-engine NOPs
    placed immediately before the offending instruction."""
    n = 0
    for f in nc.m.functions:
        for bb in f.blocks:
            insts = bb.instructions
            out = []
            changed = False
            for ins in insts:
                si = ins.sync_info
                waits = list(si.on_wait or []) if si is not None else []
                while len(waits) > MAX_WAITS_PER_INST:
                    take = waits[:MAX_WAITS_PER_INST]
                    waits = waits[MAX_WAITS_PER_INST:]
                    nop = mybir.InstNoOp(name=f"I-waitsplit-{n}", ins=[], outs=[])
                    n += 1
                    nop.engine = ins.engine
                    nop.sync_info = mybir.SyncInfo(on_wait=take, on_update=[])
                    out.append(nop)
                    changed = True
                if changed and si is not None:
                    si.on_wait = waits
                out.append(ins)
            if changed:
                bb.instructions = out
    return n


def _patch_matmuls(nc, ps_tiles, stat_t, mov_t, first, last):
    """4 DoubleRow matmuls: one [512 x 256] patch contribution of one
    256-sample chunk.  stat_t/mov_t: [P, 2, cols] SBUF slices."""
    for r in range(4):
        nc.tensor.matmul(
            ps_tiles[r][:, 0:CW],
            lhsT=stat_t[:, :, r * P : (r + 1) * P],
            rhs=mov_t,
            start=first,
            stop=last,
            perf_mode=DR,
        )


def build_program(sim_mode: bool = False):
    nc = bass.Bass(
        "TRN2", target_bir_lowering=False, debug=False, num_devices=N_CORES
    )
    # host-packed fp8 streams: [p, chunk, ktile, col]; sample = 256*chunk +
    # 128*ktile + p; cols 0:512 = patch-row block R (stationary), 512:768 =
    # patch-col block C (moving).
    qp = nc.dram_tensor("qp", [P, NCH, 2, FP], FP8, kind="ExternalInput").ap()
    kp = nc.dram_tensor("kp", [P, NCH, 2, FP], FP8, kind="ExternalInput").ap()
    acc_out = nc.dram_tensor("acc", [P, 12], F32, kind="ExternalOutput").ap()

    with _TC(nc) as tc:
        with (
            tc.tile_pool(name="stream", bufs=1) as stream,
            tc.tile_pool(name="fin", bufs=2) as fin,
            tc.tile_pool(name="psum", bufs=1, space="PSUM") as psum,
        ):
            qt = stream.tile([P, NCH, 2, FP], FP8, name="qt")
            kt = stream.tile([P, NCH, 2, FP], FP8, name="kt")
            gpc = NCH // NDMA
            # Q stream first (Gq frees its banks early), then K.
            for t_, src in ((qt, qp), (kt, kp)):
                for g in range(NDMA):
                    cs = slice(g * gpc, (g + 1) * gpc)
                    nc.sync.dma_start(out=t_[:, cs], in_=src[:, cs])

            acc = fin.tile([P, 12], F32, name="acc", bufs=1)

            # ---- phase 1: Gq into banks 0-3 ---------------------------
            psq = [
                psum.tile([P, 512], F32, name=f"psq{r}", tag=f"bankA{r}")
                for r in range(4)
            ]
            for G in range(NCH):
                _patch_matmuls(
                    nc, psq, qt[:, G], qt[:, G, :, RW:FP], G == 0, G == NCH - 1
                )
            for r in range(4):
                sq = fin.tile([P, CW], F32, name=f"sqq{r}", tag="sq")
                nc.scalar.activation(
                    sq, psq[r][:, 0:CW], ACTF.Square,
                    accum_out=acc[:, r : r + 1],
                )

            # ---- phase 2: Gk into banks 4-7, Gx reuses banks 0-3 ------
            psk = [
                psum.tile([P, 512], F32, name=f"psk{r}", tag=f"bankB{r}")
                for r in range(4)
            ]
            psx = [
                psum.tile([P, 512], F32, name=f"psx{r}", tag=f"bankA{r}")
                for r in range(4)
            ]
            for G in range(NCH):
                _patch_matmuls(
                    nc, psk, kt[:, G], kt[:, G, :, RW:FP], G == 0, G == NCH - 1
                )
                _patch_matmuls(
                    nc, psx, qt[:, G], kt[:, G, :, RW:FP], G == 0, G == NCH - 1
                )

            # ---- tail: Gk squares on ACT, Gx squares on DVE -----------
            for r in range(4):
                sq = fin.tile([P, CW], F32, name=f"sqk{r}", tag="sq")
                nc.scalar.activation(
                    sq, psk[r][:, 0:CW], ACTF.Square,
                    accum_out=acc[:, 4 + r : 5 + r],
                )
                # DVE cannot read two PSUM operands: copy to SBUF first.
                cx = fin.tile([P, CW], F32, name=f"cx{r}", tag="cx")
                nc.vector.tensor_copy(cx, psx[r][:, 0:CW])
                sx = fin.tile([P, CW], F32, name=f"sqx{r}", tag="sx")
                nc.vector.scalar_tensor_tensor(
                    sx, cx, 1.0, cx,
                    op0=ALU.mult, op1=ALU.mult,
                    accum_out=acc[:, 8 + r : 9 + r],
                )
            nc.sync.dma_start(out=acc_out, in_=acc)

    split_excess_waits(nc)
    return nc


_CACHE = {}


def _pack(m8, rsl, csl):
    """[N, D] fp8 -> [P, NCH, 2, FP] stream layout for one core."""
    sub = np.concatenate([m8[:, rsl], m8[:, csl]], axis=1)
    return np.ascontiguousarray(
        sub.reshape(NCH, 2, P, FP).transpose(2, 0, 1, 3)
    )


def kernel(feat_q: np.ndarray, feat_k: np.ndarray) -> np.ndarray:
    import ml_dtypes

    fq = np.asarray(feat_q, dtype=np.float32)
    fk = np.asarray(feat_k, dtype=np.float32)
    assert fq.shape == (N, D) and fk.shape == (N, D)

    if "nc" not in _CACHE:
        _CACHE["nc"] = build_program()
    nc = _CACHE["nc"]

    e4 = ml_dtypes.float8_e4m3
    qn = fq / np.linalg.norm(fq, axis=1, keepdims=True)
    kn = fk / np.linalg.norm(fk, axis=1, keepdims=True)
    q8 = (qn * SCALE).astype(e4)
    k8 = (kn * SCALE).astype(e4)

    in_maps = []
    for c in range(N_CORES):
        a, b = divmod(c, 4)
        rsl = slice(a * RW, (a + 1) * RW)
        csl = slice(b * CW, (b + 1) * CW)
        in_maps.append({"qp": _pack(q8, rsl, csl), "kp": _pack(k8, rsl, csl)})
    res = run_bass_kernel_spmd(nc, in_maps, list(range(N_CORES)))
    total = np.float64(0.0)
    for c in range(N_CORES):
        a = res.results[c]["acc"].astype(np.float64)
        total += np.sum(a[:, 0:8]) - 2.0 * np.sum(a[:, 8:12])
    loss = total / (np.float64(N) * (N - 1)) / np.float64(SCALE) ** 4
    return np.asarray(loss, dtype=np.float32)


if __name__ == "__main__":
    rng = np.random.default_rng(0)
    q = rng.standard_normal((N, D)).astype(np.float32)
    k = rng.standard_normal((N, D)).astype(np.float32)
    got = kernel(q, k)
    qn = q / np.linalg.norm(q, axis=1, keepdims=True)
    kn = k / np.linalg.norm(k, axis=1, keepdims=True)
    Gq = qn.T @ qn
    Gk = kn.T @ kn
    Gx = qn.T @ kn
    want = (np.sum(Gq * Gq) + np.sum(Gk * Gk) - 2 * np.sum(Gx * Gx)) / (
        N * (N - 1)
    )
    print("loss:", got, "want:", want, "rel:", abs(got - want) / abs(want))


# revision 10
# speedup vs baseline: 4.3172x; 1.0209x over previous
"""Trainium2 Bass kernel for nn_ConstLoss_22746146800082 (fp8 Gram factorization).

loss = mean_{i != j} (Cq[i,j] - Ck[i,j])^2 with Cx the NxN pairwise cosine
matrix of feat_x (N=4096, D=1024).  With unit rows (eps terms cancel, diag
cancels exactly):

    loss*N*(N-1) = ||Gq||_F^2 + ||Gk||_F^2 - 2 ||Gx||_F^2

with the DxD feature Grams Gq = Qn^T Qn, Gk = Kn^T Kn, Gx = Qn^T Kn - 2.7x
fewer MACs than the NxN route.  Host combines the three norms.

Sharding: each 1024x1024 Gram is partitioned into a 2x4 grid of [512 x 256]
patches, one patch triple (Gq/Gk/Gx) per core.  Each core streams only its
column footprint (R 512 cols + C 256 cols, fp8, rows = all 4096 samples) of
Q and K; the matmul stationary operands are SBUF slices of the same stream
(no separate stationary DMA).  fp8e4 DoubleRow matmuls (K=256/pass)
accumulate each patch over 16 sample-chunks.  PSUM has 8 banks but there
are 12 accumulation groups, so banks ping-pong: Gq uses banks 0-3 (Q stream
arrives first), is squared out on ACT, then Gx reuses those banks while Gk
runs in banks 4-7.  Tail squares use fused square+row-sum on ACT (Gq, Gk)
and DVE scalar_tensor_tensor (Gx); the kernel DMAs out [128, 12]
per-partition partials and the host does the final weighted sum - no
on-device collectives.

Host prep: rows are L2-normalized in fp32, scaled by 16, quantized to
e4m3 (measured end-to-end rel err ~1.6e-3 vs fp64, gate is 2e-2).
"""

import numpy as np

import concourse.bass as bass
import concourse.mybir as mybir
import concourse.tile as tile
from concourse.vector_clock import ScopedClock
from concourse.bass_utils import run_bass_kernel_spmd

N_CORES = 8
N = 4096
D = 1024
P = 128

NCH = 16          # contraction chunks of 256 samples (2 DoubleRow k-tiles)
FP = 768          # per-core column footprint: R (512) | C (256)
RW = 512          # patch rows (stationary cols)
CW = 256          # patch cols (moving cols)
NDMA = 4          # stream DMA instructions per matrix (4 chunks each)
SCALE = 16.0

F32 = mybir.dt.float32
FP8 = mybir.dt.float8e4
DR = mybir.MatmulPerfMode.DoubleRow
ACTF = mybir.ActivationFunctionType
ALU = mybir.AluOpType


class _TC(tile.TileContext):
    """TileContext whose kernel-tail drain splits its semaphore waits across
    preceding sync-engine NOPs: this container's walrus build rejects a Drain
    carrying more than one sync wait ("Too many sync wait commands")."""

    def _drain_and_barrier(self, tick_clock, wait_clock):
        nc = self.nc
        probe = nc.sync.nop(nofuse=True)
        wait_clock.add_sem_waits(
            probe.ins, ScopedClock({None: tick_clock.global_clock})
        )
        waits = list(probe.ins.sync_info.on_wait or []) if probe.ins.sync_info else []
        if probe.ins.sync_info is not None:
            probe.ins.sync_info.on_wait = waits[:1]
        for w in waits[1:]:
            n2 = nc.sync.nop(nofuse=True)
            n2.ins.sync_info = mybir.SyncInfo(on_wait=[w], on_update=[])
        nc.sync.drain()
        nc.all_engine_barrier()
        popped = nc._tile_sem_poison_stack.pop()
        assert popped is self._sem_poison
        nc.clear_and_free_semaphores(list(self.sems.allocated().values()))
        nc.all_engine_barrier()


MAX_WAITS_PER_INST = 1


def split_excess_waits(nc):
    """walrus (this build) rejects instructions carrying more than a couple
    of semaphore waits.  Hoist excess waits onto injected same-engine NOPs
    placed immediately before the offending instruction."""
    n = 0
    for f in nc.m.functions:
        for bb in f.blocks:
            insts = bb.instructions
            out = []
            changed = False
            for ins in insts:
                si = ins.sync_info
                waits = list(si.on_wait or []) if si is not None else []
                while len(waits) > MAX_WAITS_PER_INST:
                    take = waits[:MAX_WAITS_PER_INST]
                    waits = waits[MAX_WAITS_PER_INST:]
                    nop = mybir.InstNoOp(name=f"I-waitsplit-{n}", ins=[], outs=[])
                    n += 1
                    nop.engine = ins.engine
                    nop.sync_info = mybir.SyncInfo(on_wait=take, on_update=[])
                    out.append(nop)
                    changed = True
                if changed and si is not None:
                    si.on_wait = waits
                out.append(ins)
            if changed:
                bb.instructions = out
    return n


def _patch_matmuls(nc, ps_tiles, stat_t, mov_t, first, last):
    """4 DoubleRow matmuls: one [512 x 256] patch contribution of one
    256-sample chunk.  stat_t/mov_t: [P, 2, cols] SBUF slices."""
    for r in range(4):
        nc.tensor.matmul(
            ps_tiles[r][:, 0:CW],
            lhsT=stat_t[:, :, r * P : (r + 1) * P],
            rhs=mov_t,
            start=first,
            stop=last,
            perf_mode=DR,
        )


def build_program(sim_mode: bool = False):
    nc = bass.Bass(
        "TRN2", target_bir_lowering=False, debug=False, num_devices=N_CORES
    )
    # host-packed fp8 streams: [p, chunk, ktile, col]; sample = 256*chunk +
    # 128*ktile + p; cols 0:512 = patch-row block R (stationary), 512:768 =
    # patch-col block C (moving).
    qp = nc.dram_tensor("qp", [P, NCH, 2, FP], FP8, kind="ExternalInput").ap()
    kp = nc.dram_tensor("kp", [P, NCH, 2, FP], FP8, kind="ExternalInput").ap()
    # per-engine partial outputs (separate DMAs so their completion chains
    # overlap): Gq + Gk squares from ACT, Gx squares from DVE
    accq_out = nc.dram_tensor("accq", [P, 4], F32, kind="ExternalOutput").ap()
    acck_out = nc.dram_tensor("acck", [P, 4], F32, kind="ExternalOutput").ap()
    accx_out = nc.dram_tensor("accx", [P, 4], F32, kind="ExternalOutput").ap()

    with _TC(nc) as tc:
        with (
            tc.tile_pool(name="stream", bufs=1) as stream,
            tc.tile_pool(name="fin", bufs=2) as fin,
            tc.tile_pool(name="psum", bufs=1, space="PSUM") as psum,
        ):
            qt = stream.tile([P, NCH, 2, FP], FP8, name="qt")
            kt = stream.tile([P, NCH, 2, FP], FP8, name="kt")
            # Q stream first (Gq frees its banks early), then K.  The last
            # K chunks ship in single-chunk DMAs so the final matmul batch
            # starts as early as possible.
            qsplit = [(0, 4), (4, 8), (8, 12), (12, 16)]
            ksplit = [(0, 4), (4, 8), (8, 12), (12, 14), (14, 15), (15, 16)]
            for t_, src, split in ((qt, qp, qsplit), (kt, kp, ksplit)):
                for lo, hi in split:
                    cs = slice(lo, hi)
                    nc.sync.dma_start(out=t_[:, cs], in_=src[:, cs])

            accq = fin.tile([P, 4], F32, name="accq", bufs=1)
            acck = fin.tile([P, 4], F32, name="acck", bufs=1)
            accx = fin.tile([P, 4], F32, name="accx", bufs=1)

            # ---- phase 1: Gq into banks 0-3 ---------------------------
            psq = [
                psum.tile([P, 512], F32, name=f"psq{r}", tag=f"bankA{r}")
                for r in range(4)
            ]
            for G in range(NCH):
                _patch_matmuls(
                    nc, psq, qt[:, G], qt[:, G, :, RW:FP], G == 0, G == NCH - 1
                )
            for r in range(4):
                sq = fin.tile([P, CW], F32, name=f"sqq{r}", tag="sq")
                nc.scalar.activation(
                    sq, psq[r][:, 0:CW], ACTF.Square,
                    accum_out=accq[:, r : r + 1],
                )
            nc.sync.dma_start(out=accq_out, in_=accq)

            # ---- phase 2: Gk into banks 4-7, Gx reuses banks 0-3 ------
            psk = [
                psum.tile([P, 512], F32, name=f"psk{r}", tag=f"bankB{r}")
                for r in range(4)
            ]
            psx = [
                psum.tile([P, 512], F32, name=f"psx{r}", tag=f"bankA{r}")
                for r in range(4)
            ]
            # Gx before Gk within each chunk: the DVE tail (copy+square) is
            # longer than the ACT tail, so let Gx stop first.
            for G in range(NCH):
                _patch_matmuls(
                    nc, psx, qt[:, G], kt[:, G, :, RW:FP], G == 0, G == NCH - 1
                )
                _patch_matmuls(
                    nc, psk, kt[:, G], kt[:, G, :, RW:FP], G == 0, G == NCH - 1
                )

            # ---- tail: Gx squares on DVE, Gk squares on ACT -----------
            # DVE cannot read two PSUM operands: copy to SBUF (bf16 - packed
            # 2-byte operands unlock the fast DVE mode; Gram-entry precision
            # loss is negligible for the final sum of squares).
            BF16 = mybir.dt.bfloat16
            for r in range(4):
                cx = fin.tile([P, CW], BF16, name=f"cx{r}", tag="cx")
                nc.vector.tensor_copy(cx, psx[r][:, 0:CW])
                sx = fin.tile([P, CW], BF16, name=f"sqx{r}", tag="sx")
                nc.vector.scalar_tensor_tensor(
                    sx, cx, 1.0, cx,
                    op0=ALU.mult, op1=ALU.mult,
                    accum_out=accx[:, r : r + 1],
                )
                sq = fin.tile([P, CW], F32, name=f"sqk{r}", tag="sq")
                nc.scalar.activation(
                    sq, psk[r][:, 0:CW], ACTF.Square,
                    accum_out=acck[:, r : r + 1],
                )
            nc.sync.dma_start(out=accx_out, in_=accx)
            nc.sync.dma_start(out=acck_out, in_=acck)

    split_excess_waits(nc)
    return nc


_CACHE = {}


def _pack(m8, rsl, csl):
    """[N, D] fp8 -> [P, NCH, 2, FP] stream layout for one core."""
    sub = np.concatenate([m8[:, rsl], m8[:, csl]], axis=1)
    return np.ascontiguousarray(
        sub.reshape(NCH, 2, P, FP).transpose(2, 0, 1, 3)
    )


def kernel(feat_q: np.ndarray, feat_k: np.ndarray) -> np.ndarray:
    import ml_dtypes

    fq = np.asarray(feat_q, dtype=np.float32)
    fk = np.asarray(feat_k, dtype=np.float32)
    assert fq.shape == (N, D) and fk.shape == (N, D)

    if "nc" not in _CACHE:
        _CACHE["nc"] = build_program()
    nc = _CACHE["nc"]

    e4 = ml_dtypes.float8_e4m3
    qn = fq / np.linalg.norm(fq, axis=1, keepdims=True)
    kn = fk / np.linalg.norm(fk, axis=1, keepdims=True)
    q8 = (qn * SCALE).astype(e4)
    k8 = (kn * SCALE).astype(e4)

    in_maps = []
    for c in range(N_CORES):
        a, b = divmod(c, 4)
        rsl = slice(a * RW, (a + 1) * RW)
        csl = slice(b * CW, (b + 1) * CW)
        in_maps.append({"qp": _pack(q8, rsl, csl), "kp": _pack(k8, rsl, csl)})
    res = run_bass_kernel_spmd(nc, in_maps, list(range(N_CORES)))
    total = np.float64(0.0)
    for c in range(N_CORES):
        r = res.results[c]
        total += (
            np.sum(r["accq"].astype(np.float64))
            + np.sum(r["acck"].astype(np.float64))
            - 2.0 * np.sum(r["accx"].astype(np.float64))
        )
    loss = total / (np.float64(N) * (N - 1)) / np.float64(SCALE) ** 4
    return np.asarray(loss, dtype=np.float32)


if __name__ == "__main__":
    rng = np.random.default_rng(0)
    q = rng.standard_normal((N, D)).astype(np.float32)
    k = rng.standard_normal((N, D)).astype(np.float32)
    got = kernel(q, k)
    qn = q / np.linalg.norm(q, axis=1, keepdims=True)
    kn = k / np.linalg.norm(k, axis=1, keepdims=True)
    Gq = qn.T @ qn
    Gk = kn.T @ kn
    Gx = qn.T @ kn
    want = (np.sum(Gq * Gq) + np.sum(Gk * Gk) - 2 * np.sum(Gx * Gx)) / (
        N * (N - 1)
    )
    print("loss:", got, "want:", want, "rel:", abs(got - want) / abs(want))


# revision 13
# speedup vs baseline: 4.7640x; 1.1035x over previous
"""Trainium2 Bass kernel for nn_ConstLoss_22746146800082 (fp8 Gram factorization).

loss = mean_{i != j} (Cq[i,j] - Ck[i,j])^2 with Cx the NxN pairwise cosine
matrix of feat_x (N=4096, D=1024).  With unit rows (eps terms cancel, the
NxN diagonal cancels exactly):

    loss*N*(N-1) = ||Gq||_F^2 + ||Gk||_F^2 - 2 ||Gx||_F^2

with DxD feature Grams Gq = Qn^T Qn, Gk = Kn^T Kn, Gx = Qn^T Kn - 2.7x
fewer MACs than the NxN route.

Sharding exploits Gram symmetry (||M[g,h]||^2 = ||M[h,g]||^2 for Gq/Gk, and
Gx[h,g] is computable from the same columns as Gx[g,h]): the 1024 feature
columns form 4 groups of 256; each core takes a 2-group window {a,b} and
streams ONLY those 512 columns of Q and K (4MB fp8 per core, rows = all
4096 samples).  Per core, 7 jobs - Gq[a,b], Gq[a,a], Gk[a,b], Gk[a,a],
Gx[a,b], Gx[b,a], Gx[a,a] - each a [256 x 256] block accumulated in ONE
PSUM bank (two [128 x 256] column-regions share a bank; start=True only on
the bank's first matmul, later regions initialize via the lazy 2KB
zero-region).  The 8 windows over-cover the block space, so the host
applies per-bank multiplicity weights to the returned per-partition
partials.  Stationary operands are SBUF slices of the stream (no separate
stationary DMA); all matmuls are fp8e4 DoubleRow (K=256/pass).  Q streams
first so the two Gq banks are squared out early; the five K-phase banks
are squared at the end, split across ACT (PSUM-direct square+accum) and
DVE (copy + square+accum).  No on-device collectives; the host does the
final weighted sum.

Host prep: rows are L2-normalized in fp32, scaled by 16, quantized to
e4m3 (measured end-to-end rel err ~1.8e-3 vs fp64, gate is 2e-2).
"""

import numpy as np

import concourse.bass as bass
import concourse.mybir as mybir
import concourse.tile as tile
from concourse.vector_clock import ScopedClock
from concourse.bass_utils import run_bass_kernel_spmd

N_CORES = 8
N = 4096
D = 1024
P = 128

NCH = 16          # contraction chunks of 256 samples (2 DoubleRow k-tiles)
GW = 256          # feature-group width
W = 512           # per-core window: [group a | group b]
SCALE = 16.0

# (a, b) window per core; slot a also hosts the diagonal jobs.  Chosen so
# every group appears as some core's slot a (triangle coverage).
WINDOWS = [(0, 1), (2, 0), (3, 0), (1, 2), (3, 1), (2, 3), (1, 0), (3, 2)]

F32 = mybir.dt.float32
FP8 = mybir.dt.float8e4
DR = mybir.MatmulPerfMode.DoubleRow
ACTF = mybir.ActivationFunctionType
ALU = mybir.AluOpType


def _weights():
    """Per-core, per-bank combine weights.

    Banks: 0 Gq[a,b], 1 Gq[a,a], 2 Gk[a,b], 3 Gk[a,a], 4 Gx[a,b],
    5 Gx[b,a], 6 Gx[a,a].  Off-diag symmetric-gram blocks carry weight
    2/m_w (transpose counted via ||M||=||M^T||), Gx off-diag 1/m_w each
    (both orders computed), diagonals 1/m_a.  m_w = cores sharing the
    unordered window, m_a = cores sharing slot-a group.
    """
    from collections import Counter

    m_w = Counter(frozenset(w) for w in WINDOWS)
    m_a = Counter(a for a, _ in WINDOWS)
    wts = []
    for a, b in WINDOWS:
        mw = m_w[frozenset((a, b))]
        ma = m_a[a]
        wts.append(
            [2.0 / mw, 1.0 / ma, 2.0 / mw, 1.0 / ma, 1.0 / mw, 1.0 / mw, 1.0 / ma]
        )
    return np.array(wts, dtype=np.float64)


class _TC(tile.TileContext):
    """TileContext whose kernel-tail drain splits its semaphore waits across
    preceding sync-engine NOPs: this container's walrus build rejects a Drain
    carrying more than one sync wait ("Too many sync wait commands")."""

    def _drain_and_barrier(self, tick_clock, wait_clock):
        nc = self.nc
        probe = nc.sync.nop(nofuse=True)
        wait_clock.add_sem_waits(
            probe.ins, ScopedClock({None: tick_clock.global_clock})
        )
        waits = list(probe.ins.sync_info.on_wait or []) if probe.ins.sync_info else []
        if probe.ins.sync_info is not None:
            probe.ins.sync_info.on_wait = waits[:1]
        for w in waits[1:]:
            n2 = nc.sync.nop(nofuse=True)
            n2.ins.sync_info = mybir.SyncInfo(on_wait=[w], on_update=[])
        nc.sync.drain()
        nc.all_engine_barrier()
        popped = nc._tile_sem_poison_stack.pop()
        assert popped is self._sem_poison
        nc.clear_and_free_semaphores(list(self.sems.allocated().values()))
        nc.all_engine_barrier()


MAX_WAITS_PER_INST = 1


def split_excess_waits(nc):
    """walrus (this build) rejects instructions carrying more than a couple
    of semaphore waits.  Hoist excess waits onto injected same-engine NOPs
    placed immediately before the offending instruction."""
    n = 0
    for f in nc.m.functions:
        for bb in f.blocks:
            insts = bb.instructions
            out = []
            changed = False
            for ins in insts:
                si = ins.sync_info
                waits = list(si.on_wait or []) if si is not None else []
                while len(waits) > MAX_WAITS_PER_INST:
                    take = waits[:MAX_WAITS_PER_INST]
                    waits = waits[MAX_WAITS_PER_INST:]
                    nop = mybir.InstNoOp(name=f"I-waitsplit-{n}", ins=[], outs=[])
                    n += 1
                    nop.engine = ins.engine
                    nop.sync_info = mybir.SyncInfo(on_wait=take, on_update=[])
                    out.append(nop)
                    changed = True
                if changed and si is not None:
                    si.on_wait = waits
                out.append(ins)
            if changed:
                bb.instructions = out
    return n


def _block_matmuls(nc, bank, stat, mov, first, last, stat_off=0):
    """One [256 x 256] block contribution of one 256-sample chunk: two
    DoubleRow matmuls into the two column-regions of `bank`.  start=True
    only on the bank's very first matmul (lazy zero-region initializes the
    second region).  stat_off selects the stationary group (0 = a, GW = b)
    within the [P, 2, W] chunk tile."""
    for h in range(2):
        lo = stat_off + h * P
        nc.tensor.matmul(
            bank[:, h * GW : (h + 1) * GW],
            lhsT=stat[:, :, lo : lo + P],
            rhs=mov,
            start=first and h == 0,
            stop=last,
            perf_mode=DR,
            skip_group_check=True,
        )


def build_program(sim_mode: bool = False):
    nc = bass.Bass(
        "TRN2", target_bir_lowering=False, debug=False, num_devices=N_CORES
    )
    # host-packed fp8 window streams: [p, chunk, ktile, col]; sample =
    # 256*chunk + 128*ktile + p; cols 0:256 = group a, 256:512 = group b.
    qw = nc.dram_tensor("qw", [P, NCH, 2, W], FP8, kind="ExternalInput").ap()
    kw = nc.dram_tensor("kw", [P, NCH, 2, W], FP8, kind="ExternalInput").ap()
    # per-partition partial sums: accq = banks 0-1 (early), acct = banks 2-6
    accq_out = nc.dram_tensor("accq", [P, 2], F32, kind="ExternalOutput").ap()
    acct_out = nc.dram_tensor("acct", [P, 5], F32, kind="ExternalOutput").ap()

    with _TC(nc) as tc:
        with (
            tc.tile_pool(name="stream", bufs=1) as stream,
            tc.tile_pool(name="fin", bufs=2) as fin,
            tc.tile_pool(name="psum", bufs=1, space="PSUM") as psum,
        ):
            qt = stream.tile([P, NCH, 2, W], FP8, name="qt")
            kt = stream.tile([P, NCH, 2, W], FP8, name="kt")
            # Q first (frees its banks early); last K chunks ship in
            # single-chunk DMAs so the final matmul batch starts early.
            qsplit = [(0, 4), (4, 8), (8, 12), (12, 16)]
            ksplit = [(0, 4), (4, 8), (8, 12), (12, 14), (14, 15), (15, 16)]
            for t_, src, split in ((qt, qw, qsplit), (kt, kw, ksplit)):
                for lo, hi in split:
                    cs = slice(lo, hi)
                    nc.sync.dma_start(out=t_[:, cs], in_=src[:, cs])

            banks = [
                psum.tile([P, 512], F32, name=f"bank{i}", tag=f"bank{i}")
                for i in range(7)
            ]
            accq = fin.tile([P, 2], F32, name="accq", bufs=1)
            acct = fin.tile([P, 5], F32, name="acct", bufs=1)

            # ---- phase 1: Gq jobs from the Q stream -------------------
            for G in range(NCH):
                fl = (G == 0, G == NCH - 1)
                qa = qt[:, G, :, 0:GW]
                qb = qt[:, G, :, GW:W]
                _block_matmuls(nc, banks[0], qt[:, G], qb, *fl)   # Gq[a,b]
                _block_matmuls(nc, banks[1], qt[:, G], qa, *fl)   # Gq[a,a]
            for i in range(2):
                sq = fin.tile([P, 512], F32, name=f"sqq{i}", tag="sq")
                nc.scalar.activation(
                    sq, banks[i], ACTF.Square, accum_out=accq[:, i : i + 1]
                )
            nc.sync.dma_start(out=accq_out, in_=accq)

            # ---- phase 2: Gk + Gx jobs from the K stream --------------
            for G in range(NCH):
                fl = (G == 0, G == NCH - 1)
                ka = kt[:, G, :, 0:GW]
                kb = kt[:, G, :, GW:W]
                qs = qt[:, G]                       # q chunk tile [P, 2, W]
                _block_matmuls(nc, banks[4], qs, kb, *fl)            # Gx[a,b]
                _block_matmuls(nc, banks[5], qs, ka, *fl, stat_off=GW)  # Gx[b,a]
                _block_matmuls(nc, banks[6], qs, ka, *fl)            # Gx[a,a]
                _block_matmuls(nc, banks[2], kt[:, G], kb, *fl)      # Gk[a,b]
                _block_matmuls(nc, banks[3], kt[:, G], ka, *fl)      # Gk[a,a]

            # ---- tail: DVE banks 4,5 (copy + square), ACT banks 6,2,3 -
            for i in (4, 5):
                cx = fin.tile([P, 512], F32, name=f"cx{i}", tag="cx")
                nc.vector.tensor_copy(cx, banks[i])
                sx = fin.tile([P, 512], F32, name=f"sx{i}", tag="sx")
                nc.vector.scalar_tensor_tensor(
                    sx, cx, 1.0, cx, op0=ALU.mult, op1=ALU.mult,
                    accum_out=acct[:, i - 4 + 2 : i - 4 + 3],
                )
            for i in (6, 2, 3):
                sq = fin.tile([P, 512], F32, name=f"sqt{i}", tag="sq")
                col = {6: 4, 2: 0, 3: 1}[i]
                nc.scalar.activation(
                    sq, banks[i], ACTF.Square,
                    accum_out=acct[:, col : col + 1],
                )
            nc.sync.dma_start(out=acct_out, in_=acct)

    split_excess_waits(nc)
    return nc


_CACHE = {}


def _pack(m8, a, b):
    """[N, D] fp8 -> [P, NCH, 2, W] window stream for groups (a, b)."""
    sub = np.concatenate(
        [m8[:, a * GW : (a + 1) * GW], m8[:, b * GW : (b + 1) * GW]], axis=1
    )
    return np.ascontiguousarray(sub.reshape(NCH, 2, P, W).transpose(2, 0, 1, 3))


def kernel(feat_q: np.ndarray, feat_k: np.ndarray) -> np.ndarray:
    import ml_dtypes

    fq = np.asarray(feat_q, dtype=np.float32)
    fk = np.asarray(feat_k, dtype=np.float32)
    assert fq.shape == (N, D) and fk.shape == (N, D)

    if "nc" not in _CACHE:
        _CACHE["nc"] = build_program()
    nc = _CACHE["nc"]

    e4 = ml_dtypes.float8_e4m3
    qn = fq / np.linalg.norm(fq, axis=1, keepdims=True)
    kn = fk / np.linalg.norm(fk, axis=1, keepdims=True)
    q8 = (qn * SCALE).astype(e4)
    k8 = (kn * SCALE).astype(e4)

    in_maps = [
        {"qw": _pack(q8, a, b), "kw": _pack(k8, a, b)} for a, b in WINDOWS
    ]
    res = run_bass_kernel_spmd(nc, in_maps, list(range(N_CORES)))

    wts = _weights()
    total = np.float64(0.0)
    for c in range(N_CORES):
        r = res.results[c]
        # bank partial sums: accq = [Gq_ab, Gq_aa]; acct = [Gk_ab, Gk_aa,
        # Gx_ab, Gx_ba, Gx_aa]
        pq = np.sum(r["accq"].astype(np.float64), axis=0)
        pt = np.sum(r["acct"].astype(np.float64), axis=0)
        vals = np.array(
            [pq[0], pq[1], pt[0], pt[1], pt[2], pt[3], pt[4]], dtype=np.float64
        )
        sgn = np.array([1.0, 1.0, 1.0, 1.0, -2.0, -2.0, -2.0])
        total += np.sum(wts[c] * sgn * vals)
    loss = total / (np.float64(N) * (N - 1)) / np.float64(SCALE) ** 4
    return np.asarray(loss, dtype=np.float32)


if __name__ == "__main__":
    rng = np.random.default_rng(0)
    q = rng.standard_normal((N, D)).astype(np.float32)
    k = rng.standard_normal((N, D)).astype(np.float32)
    got = kernel(q, k)
    qn = q / np.linalg.norm(q, axis=1, keepdims=True)
    kn = k / np.linalg.norm(k, axis=1, keepdims=True)
    Gq = qn.T @ qn
    Gk = kn.T @ kn
    Gx = qn.T @ kn
    want = (np.sum(Gq * Gq) + np.sum(Gk * Gk) - 2 * np.sum(Gx * Gx)) / (
        N * (N - 1)
    )
    print("loss:", got, "want:", want, "rel:", abs(got - want) / abs(want))


# revision 14
# speedup vs baseline: 5.6476x; 1.1855x over previous
"""Trainium2 Bass kernel for nn_ConstLoss_22746146800082 (fp8 Gram factorization).

loss = mean_{i != j} (Cq[i,j] - Ck[i,j])^2 with Cx the NxN pairwise cosine
matrix of feat_x (N=4096, D=1024).  With unit rows (eps terms cancel, the
NxN diagonal cancels exactly):

    loss*N*(N-1) = ||Gq||_F^2 + ||Gk||_F^2 - 2 ||Gx||_F^2

with DxD feature Grams Gq = Qn^T Qn, Gk = Kn^T Kn, Gx = Qn^T Kn - 2.7x
fewer MACs than the NxN route.

Block cover: the 1024 feature columns form 4 groups of 256; the needed
norms decompose into 36 inter-group blocks (10 Gq + 10 Gk + 16 Gx, using
||M|| = ||M^T|| to keep one orientation of each symmetric pair).  Each core
hosts FIVE generic block-slots over two host-packed column streams
S1 = [s1a|s1b], S2 = [s2a|s2b] (512 cols each, all 4096 rows, fp8):

    B0 = s1a^T s1a   B1 = s1a^T s1b   B2 = s2a^T s2b
    B3 = s1a^T s2a   B4 = s1b^T s2b

With q-core g: S1 = [qg|qg+1], S2 = [kg|kg+2] and k-core g: S1 = [kg|kg+1],
S2 = [qg+1|qg+3] (cores 0-3 / 4-7, group indices mod 4), the 40 slots cover
all 36 blocks and the combine weights are the SAME for every core:
total = sum_cores b0 + 2*b1 + b2 - 2*b3 - 2*b4.  40 slots vs 36 needed =
90% PE efficiency, 4MB DMA per core, 5 PSUM banks.

Each block accumulates [256 x 256] in one PSUM bank (two [128 x 256]
regions share the bank; start=True only on the bank's first matmul, the
second region initializes via the lazy 2KB zero-region).  All matmuls are
fp8e4 DoubleRow (K=256/pass, 0.5 cyc/row).  S1 streams first: B0/B1 finish
early and are squared out (fused square+row-sum) on ACT while S2 streams;
the final tail is only banks 2,3 (ACT) and 4 (DVE).  Warmup + gap-filler
matmuls into a scratch bank keep the PE continuously busy so it reaches
its full-speed p-state and stays there.  No on-device collectives; the
host sums 8x128x5 partials with fixed weights.

Host prep: rows are L2-normalized in fp32, scaled by 16, quantized to
e4m3 (measured end-to-end rel err ~1.8e-3 vs fp64, gate is 2e-2).
"""

import numpy as np

import concourse.bass as bass
import concourse.mybir as mybir
import concourse.tile as tile
from concourse.vector_clock import ScopedClock
from concourse.bass_utils import run_bass_kernel_spmd

N_CORES = 8
N = 4096
D = 1024
P = 128

NCH = 16          # contraction chunks of 256 samples (2 DoubleRow k-tiles)
GW = 256          # feature-group width
W = 512           # stream width: two groups
SCALE = 16.0
NWARM = 28        # PE p-state warmup matmuls before the first stream chunk
PAD1 = 3          # gap-filler matmuls per chunk, S1 phase (PE underfed)
PAD2 = 1          # gap-filler matmuls per chunk, S2 phase

F32 = mybir.dt.float32
FP8 = mybir.dt.float8e4
DR = mybir.MatmulPerfMode.DoubleRow
ACTF = mybir.ActivationFunctionType
ALU = mybir.AluOpType


class _TC(tile.TileContext):
    """TileContext whose kernel-tail drain splits its semaphore waits across
    preceding sync-engine NOPs: this container's walrus build rejects a Drain
    carrying more than one sync wait ("Too many sync wait commands")."""

    def _drain_and_barrier(self, tick_clock, wait_clock):
        nc = self.nc
        probe = nc.sync.nop(nofuse=True)
        wait_clock.add_sem_waits(
            probe.ins, ScopedClock({None: tick_clock.global_clock})
        )
        waits = list(probe.ins.sync_info.on_wait or []) if probe.ins.sync_info else []
        if probe.ins.sync_info is not None:
            probe.ins.sync_info.on_wait = waits[:1]
        for w in waits[1:]:
            n2 = nc.sync.nop(nofuse=True)
            n2.ins.sync_info = mybir.SyncInfo(on_wait=[w], on_update=[])
        nc.sync.drain()
        nc.all_engine_barrier()
        popped = nc._tile_sem_poison_stack.pop()
        assert popped is self._sem_poison
        nc.clear_and_free_semaphores(list(self.sems.allocated().values()))
        nc.all_engine_barrier()


MAX_WAITS_PER_INST = 1


def split_excess_waits(nc):
    """walrus (this build) rejects instructions carrying more than a couple
    of semaphore waits.  Hoist excess waits onto injected same-engine NOPs
    placed immediately before the offending instruction."""
    n = 0
    for f in nc.m.functions:
        for bb in f.blocks:
            insts = bb.instructions
            out = []
            changed = False
            for ins in insts:
                si = ins.sync_info
                waits = list(si.on_wait or []) if si is not None else []
                while len(waits) > MAX_WAITS_PER_INST:
                    take = waits[:MAX_WAITS_PER_INST]
                    waits = waits[MAX_WAITS_PER_INST:]
                    nop = mybir.InstNoOp(name=f"I-waitsplit-{n}", ins=[], outs=[])
                    n += 1
                    nop.engine = ins.engine
                    nop.sync_info = mybir.SyncInfo(on_wait=take, on_update=[])
                    out.append(nop)
                    changed = True
                if changed and si is not None:
                    si.on_wait = waits
                out.append(ins)
            if changed:
                bb.instructions = out
    return n


def build_program(sim_mode: bool = False):
    nc = bass.Bass(
        "TRN2", target_bir_lowering=False, debug=False, num_devices=N_CORES
    )
    # host-packed fp8 streams: [p, chunk, ktile, col]; sample = 256*chunk +
    # 128*ktile + p; cols 0:256 = group slot a, 256:512 = slot b.
    s1 = nc.dram_tensor("s1", [P, NCH, 2, W], FP8, kind="ExternalInput").ap()
    s2 = nc.dram_tensor("s2", [P, NCH, 2, W], FP8, kind="ExternalInput").ap()
    # per-partition partials: acc1 = banks 0,1 (early), acc2 = banks 2,3,4
    acc1_out = nc.dram_tensor("acc1", [P, 2], F32, kind="ExternalOutput").ap()
    acc2_out = nc.dram_tensor("acc2", [P, 3], F32, kind="ExternalOutput").ap()

    with _TC(nc) as tc:
        with (
            tc.tile_pool(name="stream", bufs=1) as stream,
            tc.tile_pool(name="fin", bufs=2) as fin,
            tc.tile_pool(name="psum", bufs=1, space="PSUM") as psum,
        ):
            t1 = stream.tile([P, NCH, 2, W], FP8, name="t1")
            t2 = stream.tile([P, NCH, 2, W], FP8, name="t2")
            # S1 first (banks 0,1 finish + square out early); last S2 chunks
            # ship in single-chunk DMAs so the final matmuls start early.
            split1 = [(0, 4), (4, 8), (8, 12), (12, 16)]
            split2 = [(0, 4), (4, 8), (8, 12), (12, 14), (14, 15), (15, 16)]
            for t_, src, split in ((t1, s1, split1), (t2, s2, split2)):
                for lo, hi in split:
                    cs = slice(lo, hi)
                    nc.sync.dma_start(out=t_[:, cs], in_=src[:, cs])

            banks = [
                psum.tile([P, 512], F32, name=f"bank{i}", tag=f"bank{i}")
                for i in range(5)
            ]
            scratch = psum.tile([P, 512], F32, name="scratch", tag="scratch")
            acc1 = fin.tile([P, 2], F32, name="acc1", bufs=1)
            acc2 = fin.tile([P, 3], F32, name="acc2", bufs=1)

            # warmup tile: memset once, then independent matmuls keep the PE
            # busy (and ramping to full p-state) until the stream arrives.
            wz = fin.tile([P, 2, GW], FP8, name="wz", bufs=1)
            nc.vector.memset(wz, 0.25)

            def pad_mm(src_tile, G):
                nc.tensor.matmul(
                    scratch[:, 256:512],
                    lhsT=src_tile[:, G, :, 0:P],
                    rhs=src_tile[:, G, :, 0:GW],
                    start=True,
                    stop=True,
                    perf_mode=DR,
                    skip_group_check=True,
                )

            def block_mm(bank, stat_t, stat_off, mov, first, last):
                for h in range(2):
                    lo = stat_off + h * P
                    nc.tensor.matmul(
                        bank[:, h * GW : (h + 1) * GW],
                        lhsT=stat_t[:, :, lo : lo + P],
                        rhs=mov,
                        start=first and h == 0,
                        stop=last,
                        perf_mode=DR,
                        skip_group_check=True,
                    )

            for i in range(NWARM):
                nc.tensor.matmul(
                    scratch[:, 0:GW],
                    lhsT=wz[:, :, 0:P],
                    rhs=wz,
                    start=True,
                    stop=True,
                    perf_mode=DR,
                    skip_group_check=True,
                )

            # ---- S1 phase: B0 = s1a^T s1a, B1 = s1a^T s1b --------------
            for G in range(NCH):
                fl = (G == 0, G == NCH - 1)
                c1 = t1[:, G]
                a1 = t1[:, G, :, 0:GW]
                b1 = t1[:, G, :, GW:W]
                block_mm(banks[0], c1, 0, a1, *fl)
                block_mm(banks[1], c1, 0, b1, *fl)
                for _ in range(PAD1):
                    pad_mm(t1, G)
            for i in range(2):
                sq = fin.tile([P, 512], F32, name=f"sq{i}", tag="sq")
                nc.scalar.activation(
                    sq, banks[i], ACTF.Square, accum_out=acc1[:, i : i + 1]
                )
            nc.sync.dma_start(out=acc1_out, in_=acc1)

            # ---- S2 phase: B2 = s2a^T s2b, B3 = s1a^T s2a, -------------
            # ----           B4 = s1b^T s2b                  -------------
            for G in range(NCH):
                fl = (G == 0, G == NCH - 1)
                a2 = t2[:, G, :, 0:GW]
                b2 = t2[:, G, :, GW:W]
                block_mm(banks[2], t2[:, G], 0, b2, *fl)
                block_mm(banks[3], t1[:, G], 0, a2, *fl)
                block_mm(banks[4], t1[:, G], GW, b2, *fl)
                for _ in range(PAD2):
                    pad_mm(t2, G)

            # ---- tail: banks 2,3 on ACT, bank 4 on DVE -----------------
            for col, i in ((0, 2), (1, 3)):
                sq = fin.tile([P, 512], F32, name=f"sqt{i}", tag="sq")
                nc.scalar.activation(
                    sq, banks[i], ACTF.Square, accum_out=acc2[:, col : col + 1]
                )
            cx = fin.tile([P, 512], F32, name="cx", tag="cx")
            nc.vector.tensor_copy(cx, banks[4])
            sx = fin.tile([P, 512], F32, name="sx", tag="sx")
            nc.vector.scalar_tensor_tensor(
                sx, cx, 1.0, cx, op0=ALU.mult, op1=ALU.mult,
                accum_out=acc2[:, 2:3],
            )
            nc.sync.dma_start(out=acc2_out, in_=acc2)

    split_excess_waits(nc)
    return nc


_CACHE = {}


def _pack(m8, ga, gb):
    """[N, D] fp8 -> [P, NCH, 2, W] stream with groups (ga, gb) (mod 4)."""
    ga %= 4
    gb %= 4
    sub = np.concatenate(
        [m8[:, ga * GW : (ga + 1) * GW], m8[:, gb * GW : (gb + 1) * GW]], axis=1
    )
    return np.ascontiguousarray(sub.reshape(NCH, 2, P, W).transpose(2, 0, 1, 3))


def kernel(feat_q: np.ndarray, feat_k: np.ndarray) -> np.ndarray:
    import ml_dtypes

    fq = np.asarray(feat_q, dtype=np.float32)
    fk = np.asarray(feat_k, dtype=np.float32)
    assert fq.shape == (N, D) and fk.shape == (N, D)

    if "nc" not in _CACHE:
        _CACHE["nc"] = build_program()
    nc = _CACHE["nc"]

    e4 = ml_dtypes.float8_e4m3
    qn = fq / np.linalg.norm(fq, axis=1, keepdims=True)
    kn = fk / np.linalg.norm(fk, axis=1, keepdims=True)
    q8 = (qn * SCALE).astype(e4)
    k8 = (kn * SCALE).astype(e4)

    in_maps = []
    for g in range(4):  # q-cores
        in_maps.append({"s1": _pack(q8, g, g + 1), "s2": _pack(k8, g, g + 2)})
    for g in range(4):  # k-cores
        in_maps.append({"s1": _pack(k8, g, g + 1), "s2": _pack(q8, g + 1, g + 3)})
    res = run_bass_kernel_spmd(nc, in_maps, list(range(N_CORES)))

    # uniform weights: b0 + 2*b1 + b2 - 2*b3 - 2*b4
    total = np.float64(0.0)
    for c in range(N_CORES):
        r = res.results[c]
        a1 = np.sum(r["acc1"].astype(np.float64), axis=0)
        a2 = np.sum(r["acc2"].astype(np.float64), axis=0)
        total += a1[0] + 2.0 * a1[1] + a2[0] - 2.0 * a2[1] - 2.0 * a2[2]
    loss = total / (np.float64(N) * (N - 1)) / np.float64(SCALE) ** 4
    return np.asarray(loss, dtype=np.float32)


if __name__ == "__main__":
    rng = np.random.default_rng(0)
    q = rng.standard_normal((N, D)).astype(np.float32)
    k = rng.standard_normal((N, D)).astype(np.float32)
    got = kernel(q, k)
    qn = q / np.linalg.norm(q, axis=1, keepdims=True)
    kn = k / np.linalg.norm(k, axis=1, keepdims=True)
    Gq = qn.T @ qn
    Gk = kn.T @ kn
    Gx = qn.T @ kn
    want = (np.sum(Gq * Gq) + np.sum(Gk * Gk) - 2 * np.sum(Gx * Gx)) / (
        N * (N - 1)
    )
    print("loss:", got, "want:", want, "rel:", abs(got - want) / abs(want))


# revision 21
# speedup vs baseline: 5.8996x; 1.0446x over previous
"""Trainium2 Bass kernel for nn_ConstLoss_22746146800082 (fp8 Gram factorization).

loss = mean_{i != j} (Cq[i,j] - Ck[i,j])^2 with Cx the NxN pairwise cosine
matrix of feat_x (N=4096, D=1024).  With unit rows (eps terms cancel, the
NxN diagonal cancels exactly):

    loss*N*(N-1) = ||Gq||_F^2 + ||Gk||_F^2 - 2 ||Gx||_F^2

with DxD feature Grams Gq = Qn^T Qn, Gk = Kn^T Kn, Gx = Qn^T Kn - 2.7x
fewer MACs than the NxN route.

Block cover: the 1024 feature columns form 4 groups of 256; the needed
norms decompose into 36 inter-group blocks (10 Gq + 10 Gk + 16 Gx, using
||M|| = ||M^T|| to keep one orientation of each symmetric pair).  Each core
hosts FIVE generic block-slots over two host-packed column streams
S1 = [s1a|s1b], S2 = [s2a|s2b] (512 cols each, all 4096 rows, fp8):

    B0 = s1a^T s1a   B1 = s1a^T s1b   B2 = s2a^T s2b
    B3 = s1a^T s2a   B4 = s1b^T s2b

With q-core g: S1 = [qg|qg+1], S2 = [kg|kg+2] and k-core g: S1 = [kg|kg+1],
S2 = [qg+1|qg+3] (cores 0-3 / 4-7, group indices mod 4), the 40 slots cover
all 36 blocks and the combine weights are the SAME for every core:
total = sum_cores b0 + 2*b1 + b2 - 2*b3 - 2*b4.  40 slots vs 36 needed =
90% PE efficiency, 4MB DMA per core, 5 PSUM banks.

Each block accumulates [256 x 256] in one PSUM bank (two [128 x 256]
regions share the bank; start=True only on the bank's first matmul, the
second region initializes via the lazy 2KB zero-region).  All matmuls are
fp8e4 DoubleRow (K=256/pass, 0.5 cyc/row).  S1 streams first: B0/B1 finish
early and are squared out (fused square+row-sum) on ACT while S2 streams;
the final tail is only banks 2,3 (ACT) and 4 (DVE).  Warmup + gap-filler
matmuls into a scratch bank keep the PE continuously busy so it reaches
its full-speed p-state and stays there.  No on-device collectives; the
host sums 8x128x5 partials with fixed weights.

Host prep: rows are L2-normalized in fp32, scaled by 16, quantized to
e4m3 (measured end-to-end rel err ~1.8e-3 vs fp64, gate is 2e-2).
"""

import numpy as np

import concourse.bass as bass
import concourse.mybir as mybir
import concourse.tile as tile
from concourse.vector_clock import ScopedClock
from concourse.bass_utils import run_bass_kernel_spmd

N_CORES = 8
N = 4096
D = 1024
P = 128

NCH = 16          # contraction chunks of 256 samples (2 DoubleRow k-tiles)
GW = 256          # feature-group width
W = 512           # stream width: two groups
SCALE = 16.0
NWARM = 8         # PE p-state warmup matmuls before the first stream chunk
PAD1 = 0          # gap-filler matmuls per chunk, S1 phase
PAD2 = 0          # gap-filler matmuls per chunk, S2 phase
PAD2_LAST = 12    # no S2 pads from this chunk on (PE already lags the tail)

F32 = mybir.dt.float32
FP8 = mybir.dt.float8e4
DR = mybir.MatmulPerfMode.DoubleRow
ACTF = mybir.ActivationFunctionType
ALU = mybir.AluOpType


class _TC(tile.TileContext):
    """TileContext whose kernel-tail drain splits its semaphore waits across
    preceding sync-engine NOPs: this container's walrus build rejects a Drain
    carrying more than one sync wait ("Too many sync wait commands")."""

    def _drain_and_barrier(self, tick_clock, wait_clock):
        nc = self.nc
        probe = nc.sync.nop(nofuse=True)
        wait_clock.add_sem_waits(
            probe.ins, ScopedClock({None: tick_clock.global_clock})
        )
        waits = list(probe.ins.sync_info.on_wait or []) if probe.ins.sync_info else []
        if probe.ins.sync_info is not None:
            probe.ins.sync_info.on_wait = waits[:1]
        for w in waits[1:]:
            n2 = nc.sync.nop(nofuse=True)
            n2.ins.sync_info = mybir.SyncInfo(on_wait=[w], on_update=[])
        nc.sync.drain()
        nc.all_engine_barrier()
        popped = nc._tile_sem_poison_stack.pop()
        assert popped is self._sem_poison
        nc.clear_and_free_semaphores(list(self.sems.allocated().values()))
        nc.all_engine_barrier()


MAX_WAITS_PER_INST = 1


def split_excess_waits(nc):
    """walrus (this build) rejects instructions carrying more than a couple
    of semaphore waits.  Hoist excess waits onto injected same-engine NOPs
    placed immediately before the offending instruction."""
    n = 0
    for f in nc.m.functions:
        for bb in f.blocks:
            insts = bb.instructions
            out = []
            changed = False
            for ins in insts:
                si = ins.sync_info
                waits = list(si.on_wait or []) if si is not None else []
                while len(waits) > MAX_WAITS_PER_INST:
                    take = waits[:MAX_WAITS_PER_INST]
                    waits = waits[MAX_WAITS_PER_INST:]
                    nop = mybir.InstNoOp(name=f"I-waitsplit-{n}", ins=[], outs=[])
                    n += 1
                    nop.engine = ins.engine
                    nop.sync_info = mybir.SyncInfo(on_wait=take, on_update=[])
                    out.append(nop)
                    changed = True
                if changed and si is not None:
                    si.on_wait = waits
                out.append(ins)
            if changed:
                bb.instructions = out
    return n


def build_program(sim_mode: bool = False):
    nc = bass.Bass(
        "TRN2", target_bir_lowering=False, debug=False, num_devices=N_CORES
    )
    # host-packed fp8 streams: [p, chunk, ktile, col]; sample = 256*chunk +
    # 128*ktile + p; cols 0:256 = group slot a, 256:512 = slot b.
    s1 = nc.dram_tensor("s1", [P, NCH, 2, W], FP8, kind="ExternalInput").ap()
    s2 = nc.dram_tensor("s2", [P, NCH, 2, W], FP8, kind="ExternalInput").ap()
    # per-partition partials: acc1 = banks 0,1 (early), acc2 = banks 2,3,4
    acc1_out = nc.dram_tensor("acc1", [P, 2], F32, kind="ExternalOutput").ap()
    acc2_out = nc.dram_tensor("acc2", [P, 2], F32, kind="ExternalOutput").ap()

    with _TC(nc) as tc:
        with (
            tc.tile_pool(name="stream", bufs=1) as stream,
            tc.tile_pool(name="fin", bufs=2) as fin,
            tc.tile_pool(name="psum", bufs=1, space="PSUM") as psum,
        ):
            t1 = stream.tile([P, NCH, 2, W], FP8, name="t1")
            t2 = stream.tile([P, NCH, 2, W], FP8, name="t2")
            # S1 first (banks 0,1 finish + square out early); last S2 chunks
            # ship in single-chunk DMAs so the final matmuls start early.
            split1 = [(0, 4), (4, 8), (8, 12), (12, 16)]
            split2 = [(0, 4), (4, 8), (8, 12), (12, 14), (14, 15), (15, 16)]
            for t_, src, split in ((t1, s1, split1), (t2, s2, split2)):
                for lo, hi in split:
                    cs = slice(lo, hi)
                    nc.sync.dma_start(out=t_[:, cs], in_=src[:, cs])

            banks = [
                psum.tile([P, 512], F32, name=f"bank{i}", tag=f"bank{i}")
                for i in range(3)
            ]
            # banks 3+4 share the combine weight (-2): one contiguous 2-bank
            # allocation so the tail squares them with a single ACT pass.
            banks34 = psum.tile([P, 1024], F32, name="banks34", tag="banks34")
            banks.append(banks34[:, 0:512])
            banks.append(banks34[:, 512:1024])
            scratch = psum.tile([P, 512], F32, name="scratch", tag="scratch")
            acc1 = fin.tile([P, 2], F32, name="acc1", bufs=1)
            acc2 = fin.tile([P, 2], F32, name="acc2", bufs=1)

            # warmup tile: memset once, then independent matmuls keep the PE
            # busy (and ramping to full p-state) until the stream arrives.
            wz = fin.tile([P, 2, GW], FP8, name="wz", bufs=1)
            nc.vector.memset(wz, 0.25)

            def pad_mm(src_tile, G):
                nc.tensor.matmul(
                    scratch[:, 256:512],
                    lhsT=src_tile[:, G, :, 0:P],
                    rhs=src_tile[:, G, :, 0:GW],
                    start=True,
                    stop=True,
                    perf_mode=DR,
                    skip_group_check=True,
                )

            def block_mm(bank, stat_t, stat_off, mov, first, last):
                for h in range(2):
                    lo = stat_off + h * P
                    nc.tensor.matmul(
                        bank[:, h * GW : (h + 1) * GW],
                        lhsT=stat_t[:, :, lo : lo + P],
                        rhs=mov,
                        start=first and h == 0,
                        stop=last,
                        perf_mode=DR,
                        skip_group_check=True,
                    )

            for i in range(NWARM):
                nc.tensor.matmul(
                    scratch[:, 0:GW],
                    lhsT=wz[:, :, 0:P],
                    rhs=wz,
                    start=True,
                    stop=True,
                    perf_mode=DR,
                    skip_group_check=True,
                )

            # ---- S1 phase: B0 = s1a^T s1a, B1 = s1a^T s1b --------------
            for G in range(NCH):
                fl = (G == 0, G == NCH - 1)
                c1 = t1[:, G]
                a1 = t1[:, G, :, 0:GW]
                b1 = t1[:, G, :, GW:W]
                block_mm(banks[0], c1, 0, a1, *fl)
                block_mm(banks[1], c1, 0, b1, *fl)
                for _ in range(PAD1):
                    pad_mm(t1, G)
            for i in range(2):
                sq = fin.tile([P, 512], F32, name=f"sq{i}", tag="sq")
                nc.scalar.activation(
                    sq, banks[i], ACTF.Square, accum_out=acc1[:, i : i + 1]
                )
            nc.sync.dma_start(out=acc1_out, in_=acc1)

            # ---- S2 phase: B2 = s2a^T s2b, B3 = s1a^T s2a, -------------
            # ----           B4 = s1b^T s2b                  -------------
            for G in range(NCH):
                fl = (G == 0, G == NCH - 1)
                a2 = t2[:, G, :, 0:GW]
                b2 = t2[:, G, :, GW:W]
                block_mm(banks[2], t2[:, G], 0, b2, *fl)
                block_mm(banks[3], t1[:, G], 0, a2, *fl)
                block_mm(banks[4], t1[:, G], GW, b2, *fl)
                if G < PAD2_LAST:
                    for _ in range(PAD2):
                        pad_mm(t2, G)

            # ---- tail: banks 3+4 in one ACT pass, bank 2 on DVE --------
            cx = fin.tile([P, 512], F32, name="cx", tag="cx")
            nc.vector.tensor_copy(cx, banks[2])
            sx = fin.tile([P, 512], F32, name="sx", tag="sx")
            nc.vector.scalar_tensor_tensor(
                sx, cx, 1.0, cx, op0=ALU.mult, op1=ALU.mult,
                accum_out=acc2[:, 0:1],
            )
            sq34 = fin.tile([P, 1024], F32, name="sq34", tag="sq34")
            nc.scalar.activation(
                sq34, banks34, ACTF.Square, accum_out=acc2[:, 1:2]
            )
            nc.sync.dma_start(out=acc2_out, in_=acc2)

    split_excess_waits(nc)
    return nc


_CACHE = {}


def _pack(m8, ga, gb):
    """[N, D] fp8 -> [P, NCH, 2, W] stream with groups (ga, gb) (mod 4)."""
    ga %= 4
    gb %= 4
    sub = np.concatenate(
        [m8[:, ga * GW : (ga + 1) * GW], m8[:, gb * GW : (gb + 1) * GW]], axis=1
    )
    return np.ascontiguousarray(sub.reshape(NCH, 2, P, W).transpose(2, 0, 1, 3))


def kernel(feat_q: np.ndarray, feat_k: np.ndarray) -> np.ndarray:
    import ml_dtypes

    fq = np.asarray(feat_q, dtype=np.float32)
    fk = np.asarray(feat_k, dtype=np.float32)
    assert fq.shape == (N, D) and fk.shape == (N, D)

    if "nc" not in _CACHE:
        _CACHE["nc"] = build_program()
    nc = _CACHE["nc"]

    e4 = ml_dtypes.float8_e4m3
    qn = fq / np.linalg.norm(fq, axis=1, keepdims=True)
    kn = fk / np.linalg.norm(fk, axis=1, keepdims=True)
    q8 = (qn * SCALE).astype(e4)
    k8 = (kn * SCALE).astype(e4)

    in_maps = []
    for g in range(4):  # q-cores
        in_maps.append({"s1": _pack(q8, g, g + 1), "s2": _pack(k8, g, g + 2)})
    for g in range(4):  # k-cores
        in_maps.append({"s1": _pack(k8, g, g + 1), "s2": _pack(q8, g + 1, g + 3)})
    res = run_bass_kernel_spmd(nc, in_maps, list(range(N_CORES)))

    # uniform weights: b0 + 2*b1 + b2 - 2*(b3 + b4)
    total = np.float64(0.0)
    for c in range(N_CORES):
        r = res.results[c]
        a1 = np.sum(r["acc1"].astype(np.float64), axis=0)
        a2 = np.sum(r["acc2"].astype(np.float64), axis=0)
        total += a1[0] + 2.0 * a1[1] + a2[0] - 2.0 * a2[1]
    loss = total / (np.float64(N) * (N - 1)) / np.float64(SCALE) ** 4
    return np.asarray(loss, dtype=np.float32)


if __name__ == "__main__":
    rng = np.random.default_rng(0)
    q = rng.standard_normal((N, D)).astype(np.float32)
    k = rng.standard_normal((N, D)).astype(np.float32)
    got = kernel(q, k)
    qn = q / np.linalg.norm(q, axis=1, keepdims=True)
    kn = k / np.linalg.norm(k, axis=1, keepdims=True)
    Gq = qn.T @ qn
    Gk = kn.T @ kn
    Gx = qn.T @ kn
    want = (np.sum(Gq * Gq) + np.sum(Gk * Gk) - 2 * np.sum(Gx * Gx)) / (
        N * (N - 1)
    )
    print("loss:", got, "want:", want, "rel:", abs(got - want) / abs(want))
